# revision 23
# baseline (speedup 1.0000x reference)
"""Multi-head causal attention (B=4, T=4096, D=1024, H=16) on 8 TRN2 NeuronCores.

Sharding: core c -> (batch b = c//2, head-group g = c%2 of 8 heads).
Host sums the two per-batch partials (w_proj row-split) and transposes.

Design (measured 871us vs the 1233us v1 baseline; PE 94% busy):
  - bf16 operands everywhere (matmul rate unchanged vs fp32r, but halves
    SBUF/HBM footprint); fp32 PSUM accumulation throughout.
  - Everything SBUF-resident: K^T / V / Q^T live in SBUF between the QKV
    projection and attention -- no DRAM round-trip (v1 moved ~110MB).
  - No PE transposes: host supplies x^T, projection emits Q^T/K^T directly
    (W-stationary) and V in natural layout (x^T-stationary).
  - Causal diagonal blocks trimmed at 128-granularity: S / exp / mask / AV
    restricted to valid columns (-8% PE and ScalarE work).
  - Software pipeline: AV for tile k is emitted while tile k+1's S/exp are
    in flight, so the PE never sits on the just-issued exp. Projection and
    (deferred) output-projection units are paced into the attention stream
    as PE gap fillers; ScalarE (exp, ~580us) hides under PE (~670us).
  - softmax without max-subtraction (logits ~N(0,1)); denominator via a
    ones-column in V (comes out of the same AV matmul, PSUM row 64).
"""
import math

import numpy as np

B, T, D = 4, 4096, 1024
H, HD = 16, 64
N_CORES = 8
PAIRS = 4            # head-pairs per core (8 local heads)
DL = PAIRS * 128     # 512 = local q/k/v width
TQ = 512             # query block
NTQ = T // TQ        # 8

_CACHE = {}


def _build_nc():
    import concourse.tile as tile
    from concourse import bacc, mybir

    fp32 = mybir.dt.float32
    bf16 = mybir.dt.bfloat16
    AF = mybir.ActivationFunctionType

    nc = bacc.Bacc("TRN2", target_bir_lowering=False, debug=False,
                   num_devices=N_CORES)
    xt_d = nc.dram_tensor("xt", [D, T], bf16, kind="ExternalInput").ap()
    wqkv_d = nc.dram_tensor("wqkv", [D, 3 * DL], bf16, kind="ExternalInput").ap()
    wp_d = nc.dram_tensor("wp", [DL, D], bf16, kind="ExternalInput").ap()
    msk_d = nc.dram_tensor("msk", [128, 128], bf16, kind="ExternalInput").ap()
    yt_d = nc.dram_tensor("yt", [D, T], fp32, kind="ExternalOutput").ap()

    with tile.TileContext(nc) as tc:
        with (
            tc.tile_pool(name="sb", bufs=1) as pool,
            tc.tile_pool(name="ps", bufs=1, space="PSUM") as psum,
        ):
            # wqkv split per output-column group so the first projection
            # units only wait on their own 256KB slice, not the full 3MB.
            wqkv = pool.tile([128, 8, 3 * DL], bf16, tag="wqkv")

            def dma_wqkv(lo, hi):
                nc.sync.dma_start(
                    wqkv[:, :, lo:hi],
                    wqkv_d[:, lo:hi].rearrange("(a p) f -> p a f", p=128))

            wp = pool.tile([128, 4, D], bf16, tag="wp")
            msk = pool.tile([128, 128], bf16, tag="msk")

            # K^T and V resident for all 8 token-blocks; Q^T rotates (only
            # row j reads q block j; block j+2 is written during row j).
            kT = [pool.tile([128, PAIRS, TQ], bf16, tag=f"kT{t}",
                            name=f"kT_{t}")
                  for t in range(NTQ)]
            vb = [pool.tile([128, PAIRS, 4, 2, 65], bf16, tag=f"vb{t}",
                            name=f"vb_{t}")
                  for t in range(NTQ)]
            for t in range(NTQ):
                # ones column per head -> softmax denominator out of AV matmul
                nc.vector.memset(vb[t][:, :, :, :, 64:65], 1.0)

            qT = {}

            def dma_x(tb, split=False):
                xt = pool.tile([128, 8, TQ], bf16, tag="xt", bufs=2,
                               name=f"xt_{tb}")
                src = xt_d[:, tb * TQ:(tb + 1) * TQ]
                if split:  # halves so the first matmuls start sooner
                    nc.sync.dma_start(
                        xt[:, 0:4, :],
                        src[0:512, :].rearrange("(a p) t -> p a t", p=128))
                    nc.sync.dma_start(
                        xt[:, 4:8, :],
                        src[512:1024, :].rearrange("(a p) t -> p a t", p=128))
                else:
                    nc.sync.dma_start(
                        xt[:], src.rearrange("(a p) t -> p a t", p=128))
                return xt

            def copy_engine():
                # GPSIMD/Pool cannot read PSUM on TRN2; DVE does all
                # PSUM->SBUF drains.
                return nc.vector

            def make_units(tb, xt):
                """12 projection PE-work units for token rows [tb*512,+512)."""
                units = []

                def qk_unit(fc):
                    def run():
                        if fc == 0:
                            qT[tb] = pool.tile([128, PAIRS, TQ], bf16,
                                               tag="qt", bufs=3,
                                               name=f"qt_{tb}")
                        pp = psum.tile([128, TQ], fp32, tag="pp", bufs=2,
                                       name=f"pqk_{tb}_{fc}")
                        for kb in range(8):
                            nc.tensor.matmul(
                                pp[:], wqkv[:, kb, fc * 128:(fc + 1) * 128],
                                xt[:, kb, :], start=(kb == 0), stop=(kb == 7))
                        dst = qT[tb] if fc < 4 else kT[tb]
                        copy_engine().tensor_copy(dst[:, fc % 4, :], pp[:])
                    return run

                def v_unit(s):
                    def run():
                        pp = psum.tile([128, TQ], fp32, tag="pp", bufs=2,
                                       name=f"pv_{tb}_{s}")
                        for kb in range(8):
                            nc.tensor.matmul(
                                pp[:], xt[:, kb, s * 128:(s + 1) * 128],
                                wqkv[:, kb, 2 * DL:3 * DL],
                                start=(kb == 0), stop=(kb == 7))
                        copy_engine().tensor_copy(
                            vb[tb][:, :, s, :, 0:64],
                            pp[:].rearrange("p (a h e) -> p a h e", a=4, h=2))
                    return run

                for fc in range(8):
                    units.append(qk_unit(fc))
                for s in range(4):
                    units.append(v_unit(s))
                return units

            ob = {}  # (j, pr) -> attention-output SBUF tile [128, TQ] bf16

            def wproj_pair(j, mc0):
                """Output-proj for mc0, mc0+1, kc-major: the kc=3 matmuls
                (gated on the last pair's normalize) come last, so the PE
                isn't stalled mid-unit waiting for ob tiles."""
                def run():
                    yps = [psum.tile([128, TQ], fp32, tag="pp", bufs=2,
                                     name=f"yp_{j}_{mc0 + i}")
                           for i in range(2)]
                    for kc in range(PAIRS):
                        for i in range(2):
                            mc = mc0 + i
                            nc.tensor.matmul(
                                yps[i][:], wp[:, kc, mc * 128:(mc + 1) * 128],
                                ob[(j, kc)][:],
                                start=(kc == 0), stop=(kc == PAIRS - 1))
                    for i in range(2):
                        mc = mc0 + i
                        ys = pool.tile([128, TQ], fp32, tag="ys", bufs=2,
                                       name=f"ys_{j}_{mc}")
                        nc.vector.tensor_copy(ys[:], yps[i][:])
                        nc.sync.dma_start(
                            yt_d[mc * 128:(mc + 1) * 128, j * TQ:(j + 1) * TQ],
                            ys[:])
                return run

            # ---- attention tile pipeline (AV delayed by two tiles, so its
            # exp dependency is ~2.2us old and the semaphore-propagation
            # latency never lands on the PE's critical path) ----
            pending = []

            def do_av(j, pr, c, h, half, ot, ex):
                for tkb in range(2):
                    blk = half * 2 + tkb
                    d = 128 * blk if c == j else 0
                    nc.tensor.matmul(
                        ot[:, d:TQ], vb[c][:, pr, blk, h, :],
                        ex[:, tkb, d:TQ],
                        start=(c == 0 and half == 0 and tkb == 0),
                        stop=(c == j and half == 1 and tkb == 1))

            def flush_one():
                args, post = pending.pop(0)
                do_av(*args)
                if post is not None:
                    post()

            def flush_av():
                while pending:
                    flush_one()

            def attn_tile(j, pr, c, h, half, ot, post=None):
                st = psum.tile([128, 2, TQ], fp32, tag="st", bufs=2,
                               name=f"st_{j}_{pr}_{c}_{h}_{half}")
                for tkb in range(2):
                    blk = half * 2 + tkb
                    d = 128 * blk if c == j else 0
                    nc.tensor.matmul(
                        st[:, tkb, d:TQ],
                        kT[c][h * 64:(h + 1) * 64, pr, blk * 128:(blk + 1) * 128],
                        qT[j][h * 64:(h + 1) * 64, pr, d:TQ],
                        start=True, stop=True)
                ex = pool.tile([128, 2, TQ], bf16, tag="ex", bufs=6,
                               name=f"ex_{j}_{pr}_{c}_{h}_{half}")
                if c < j:
                    nc.scalar.activation(ex[:], st[:], AF.Exp, scale=0.125)
                else:
                    for tkb in range(2):
                        d = 128 * (half * 2 + tkb)
                        nc.scalar.activation(ex[:, tkb, d:TQ], st[:, tkb, d:TQ],
                                             AF.Exp, scale=0.125)
                        nc.vector.tensor_mul(ex[:, tkb, d:d + 128],
                                             ex[:, tkb, d:d + 128], msk[:])
                if len(pending) >= 2:
                    flush_one()
                pending.append(((j, pr, c, h, half, ot, ex), post))

            def normalize_pair(j, pr, ots):
                """h0/h1 chains interleaved across DVE and Pool."""
                den = [pool.tile([1, TQ], fp32, tag="den", bufs=2,
                                 name=f"den_{j}_{pr}_{h}") for h in range(2)]
                bc = [pool.tile([64, TQ], fp32, tag="bc", bufs=2,
                                name=f"bc_{j}_{pr}_{h}") for h in range(2)]
                rec = [pool.tile([64, TQ], fp32, tag="rec", bufs=2,
                                 name=f"rec_{j}_{pr}_{h}") for h in range(2)]
                for h in range(2):
                    nc.vector.tensor_copy(den[h][:], ots[h][64:65, :])
                for h in range(2):
                    nc.gpsimd.partition_broadcast(bc[h][:], den[h][:])
                for h in range(2):
                    nc.vector.reciprocal_approx_fast(rec[h][:], bc[h][:])
                for h in range(2):
                    nc.vector.tensor_mul(ob[(j, pr)][h * 64:(h + 1) * 64, :],
                                         ots[h][0:64, :], rec[h][:])

            # ---- main schedule ----
            # proj blocks 0,1 fully upfront; block j+2 paced across row j;
            # wproj rows 0..5 deferred and paced across rows 6..7.
            # DMA order: first units' operands first.
            dma_wqkv(0, 128)
            xts = {0: dma_x(0, split=True)}
            dma_wqkv(128, 512)
            dma_wqkv(512, 1024)
            dma_wqkv(1024, 3 * DL)
            nc.sync.dma_start(msk[:], msk_d[:])
            xts[1] = dma_x(1, split=True)
            nc.sync.dma_start(wp[:], wp_d.rearrange("(a p) f -> p a f", p=128))
            for tb in (0, 1):
                for u in make_units(tb, xts[tb]):
                    u()

            filler = []
            fill_emitted = [0]
            fill_tiles = 4 * 7 * 4 + 4 * 8 * 4  # attn tiles in rows 6+7
            fill_done = [0]

            for j in range(NTQ):
                if j + 2 < NTQ:
                    xts[j + 2] = dma_x(j + 2)
                    row_units = make_units(j + 2, xts[j + 2])
                else:
                    row_units = []
                n_units = len(row_units)
                row_tiles = 4 * (j + 1) * 4
                tcount = 0
                emitted = 0
                for pr in range(PAIRS):
                    ob[(j, pr)] = pool.tile(
                        [128, TQ], bf16,
                        tag=(f"ob{j}_{pr}" if j < 6 else "obx"),
                        bufs=(1 if j < 6 else 8),
                        name=f"ob_{j}_{pr}")
                    ot = [psum.tile([65, TQ], fp32, tag="ot", bufs=2,
                                    name=f"ot{h}_{pr}_{j}")
                          for h in range(2)]
                    for c in range(j + 1):
                        for (h, half) in ((0, 0), (1, 0), (0, 1), (1, 1)):
                            last = (c == j and h == 1 and half == 1)
                            post = None
                            if last:
                                def post(j=j, pr=pr, ots=tuple(ot)):
                                    normalize_pair(j, pr, ots)
                            attn_tile(j, pr, c, h, half, ot[h], post)
                            tcount += 1
                            target = math.ceil(n_units * tcount / row_tiles)
                            while emitted < target:
                                row_units[emitted]()
                                emitted += 1
                            if j >= 6:
                                fill_done[0] += 1
                                ft = math.ceil(
                                    len(filler) * fill_done[0] / fill_tiles)
                                while fill_emitted[0] < ft:
                                    filler[fill_emitted[0]]()
                                    fill_emitted[0] += 1
                if j <= 5:
                    for mc0 in range(0, 8, 2):
                        filler.append(wproj_pair(j, mc0))
                else:
                    flush_av()
                    for mc0 in range(0, 8, 2):
                        wproj_pair(j, mc0)()
            flush_av()
            while fill_emitted[0] < len(filler):
                filler[fill_emitted[0]]()
                fill_emitted[0] += 1

    nc.compile()
    return nc


def _get_nc():
    if "nc" not in _CACHE:
        _CACHE["nc"] = _build_nc()
    return _CACHE["nc"]


def _in_maps(x, w_qkv, w_proj):
    import ml_dtypes
    bf16 = ml_dtypes.bfloat16
    p = np.arange(128, dtype=np.int32)
    msk = (p[:, None] <= p[None, :]).astype(bf16)
    maps = []
    for c in range(N_CORES):
        b, g = c // 2, c % 2
        wq = w_qkv[:, g * DL:(g + 1) * DL]
        wk = w_qkv[:, D + g * DL:D + (g + 1) * DL]
        wv = w_qkv[:, 2 * D + g * DL:2 * D + (g + 1) * DL]
        maps.append({
            "xt": np.ascontiguousarray(x[b].T).astype(bf16),
            "wqkv": np.ascontiguousarray(
                np.concatenate([wq, wk, wv], axis=1)).astype(bf16),
            "wp": np.ascontiguousarray(w_proj[g * DL:(g + 1) * DL, :]).astype(bf16),
            "msk": msk,
        })
    return maps


def _run(x, w_qkv, w_proj, trace=False):
    from concourse.bass_utils import run_bass_kernel_spmd

    nc = _get_nc()
    res = run_bass_kernel_spmd(nc, _in_maps(x, w_qkv, w_proj),
                               core_ids=list(range(N_CORES)), trace=trace)
    outs = [res.results[c]["yt"] for c in range(N_CORES)]
    y = np.stack([(outs[2 * b] + outs[2 * b + 1]).T for b in range(B)])
    return np.ascontiguousarray(y.astype(np.float32)), res


def kernel(x, w_qkv, w_proj):
    x = np.asarray(x, dtype=np.float32)
    w_qkv = np.asarray(w_qkv, dtype=np.float32)
    w_proj = np.asarray(w_proj, dtype=np.float32)
    y, _ = _run(x, w_qkv, w_proj, trace=False)
    return y


def kernel_traced(x, w_qkv, w_proj):
    """Test-only entry: run with NTFF profiling (needs the sibling prof_shim
    module; the graded kernel() path never imports it)."""
    import prof_shim
    prof_shim.install()
    y, res = _run(np.asarray(x, np.float32), np.asarray(w_qkv, np.float32),
                  np.asarray(w_proj, np.float32), trace=True)
    return y, res


# revision 26
# speedup vs baseline: 1.0114x; 1.0114x over previous
"""Multi-head causal attention (B=4, T=4096, D=1024, H=16) on 8 TRN2 NeuronCores.

Sharding: core c -> (batch b = c//2, head-group g = c%2 of 8 heads).
Host sums the two per-batch partials (w_proj row-split) and transposes.

v2 design (vs v1 baseline at 1233us):
  - bf16 operands everywhere (matmul rate unchanged vs fp32r, but halves
    SBUF/HBM footprint); fp32 PSUM accumulation throughout.
  - Everything SBUF-resident: K^T / V / Q^T live in SBUF between the QKV
    projection and attention -- no DRAM round-trip (v1 moved ~110MB).
  - No PE transposes: host supplies x^T, projection emits Q^T/K^T directly
    (W-stationary) and V in natural layout (x^T-stationary).
  - Causal diagonal blocks trimmed at 128-granularity: S / exp / mask / AV
    restricted to valid columns (-8% PE and ScalarE work).
  - Software pipeline: AV for tile k is emitted while tile k+1's S/exp are
    in flight, so the PE never sits on the just-issued exp. Projection and
    (deferred) output-projection units are paced into the attention stream
    as PE gap fillers; ScalarE (exp, ~580us) hides under PE (~670us).
  - softmax without max-subtraction (logits ~N(0,1)); denominator via a
    ones-column in V (comes out of the same AV matmul, PSUM row 64).
"""
import math

import numpy as np

B, T, D = 4, 4096, 1024
H, HD = 16, 64
N_CORES = 8
PAIRS = 4            # head-pairs per core (8 local heads)
DL = PAIRS * 128     # 512 = local q/k/v width
TQ = 512             # query block
NTQ = T // TQ        # 8

_CACHE = {}


def _build_nc():
    import concourse.tile as tile
    from concourse import bacc, mybir

    fp32 = mybir.dt.float32
    bf16 = mybir.dt.bfloat16
    AF = mybir.ActivationFunctionType

    nc = bacc.Bacc("TRN2", target_bir_lowering=False, debug=False,
                   num_devices=N_CORES)
    xt_d = nc.dram_tensor("xt", [D, T], bf16, kind="ExternalInput").ap()
    wqkv_d = nc.dram_tensor("wqkv", [D, 3 * DL], bf16, kind="ExternalInput").ap()
    wp_d = nc.dram_tensor("wp", [DL, D], bf16, kind="ExternalInput").ap()
    msk_d = nc.dram_tensor("msk", [128, 128], bf16, kind="ExternalInput").ap()
    yt_d = nc.dram_tensor("yt", [D, T], fp32, kind="ExternalOutput").ap()

    with tile.TileContext(nc) as tc:
        with (
            tc.tile_pool(name="sb", bufs=1) as pool,
            tc.tile_pool(name="ps", bufs=1, space="PSUM") as psum,
        ):
            # wqkv split per output-column group so the first projection
            # units only wait on their own 256KB slice, not the full 3MB.
            wqkv = pool.tile([128, 8, 3 * DL], bf16, tag="wqkv")

            def dma_wqkv(lo, hi):
                nc.sync.dma_start(
                    wqkv[:, :, lo:hi],
                    wqkv_d[:, lo:hi].rearrange("(a p) f -> p a f", p=128))

            wp = pool.tile([128, 4, D], bf16, tag="wp")
            msk = pool.tile([128, 128], bf16, tag="msk")

            # K^T and V resident for all 8 token-blocks; Q^T rotates (only
            # row j reads q block j; block j+2 is written during row j).
            kT = [pool.tile([128, PAIRS, TQ], bf16, tag=f"kT{t}",
                            name=f"kT_{t}")
                  for t in range(NTQ)]
            vb = [pool.tile([128, PAIRS, 4, 2, 65], bf16, tag=f"vb{t}",
                            name=f"vb_{t}")
                  for t in range(NTQ)]
            for t in range(NTQ):
                # ones column per head -> softmax denominator out of AV matmul
                nc.vector.memset(vb[t][:, :, :, :, 64:65], 1.0)

            qT = {}

            def dma_x(tb, split=False):
                xt = pool.tile([128, 8, TQ], bf16, tag="xt", bufs=2,
                               name=f"xt_{tb}")
                src = xt_d[:, tb * TQ:(tb + 1) * TQ]
                if split:  # halves so the first matmuls start sooner
                    nc.sync.dma_start(
                        xt[:, 0:4, :],
                        src[0:512, :].rearrange("(a p) t -> p a t", p=128))
                    nc.sync.dma_start(
                        xt[:, 4:8, :],
                        src[512:1024, :].rearrange("(a p) t -> p a t", p=128))
                else:
                    nc.sync.dma_start(
                        xt[:], src.rearrange("(a p) t -> p a t", p=128))
                return xt

            def copy_engine():
                # GPSIMD/Pool cannot read PSUM on TRN2; DVE does all
                # PSUM->SBUF drains.
                return nc.vector

            # True while a projection unit's PSUM accumulation is half-open
            # (between its a/b halves). wproj pairs must not allocate pp
            # slots in that window (in-order PE + tag rotation would
            # deadlock), so the fill pacing checks this flag.
            pp_open = [False]

            def make_units(tb, xt):
                """Projection PE-work for token rows [tb*512,+512), split
                into 4-matmul halves so paced insertions between attention
                tiles never outlast the 2-deep exp queue (~2.2us)."""
                units = []

                def qk_unit(fc):
                    st_ = {}

                    def a():
                        if fc == 0:
                            qT[tb] = pool.tile([128, PAIRS, TQ], bf16,
                                               tag="qt", bufs=3,
                                               name=f"qt_{tb}")
                        st_["pp"] = psum.tile([128, TQ], fp32, tag="pp",
                                              bufs=2, name=f"pqk_{tb}_{fc}")
                        for kb in range(4):
                            nc.tensor.matmul(
                                st_["pp"][:],
                                wqkv[:, kb, fc * 128:(fc + 1) * 128],
                                xt[:, kb, :], start=(kb == 0), stop=False)
                        pp_open[0] = True

                    def b():
                        pp = st_["pp"]
                        for kb in range(4, 8):
                            nc.tensor.matmul(
                                pp[:], wqkv[:, kb, fc * 128:(fc + 1) * 128],
                                xt[:, kb, :], start=False, stop=(kb == 7))
                        dst = qT[tb] if fc < 4 else kT[tb]
                        copy_engine().tensor_copy(dst[:, fc % 4, :], pp[:])
                        pp_open[0] = False
                    return [a, b]

                def v_unit(s):
                    st_ = {}

                    def a():
                        st_["pp"] = psum.tile([128, TQ], fp32, tag="pp",
                                              bufs=2, name=f"pv_{tb}_{s}")
                        for kb in range(4):
                            nc.tensor.matmul(
                                st_["pp"][:], xt[:, kb, s * 128:(s + 1) * 128],
                                wqkv[:, kb, 2 * DL:3 * DL],
                                start=(kb == 0), stop=False)
                        pp_open[0] = True

                    def b():
                        pp = st_["pp"]
                        for kb in range(4, 8):
                            nc.tensor.matmul(
                                pp[:], xt[:, kb, s * 128:(s + 1) * 128],
                                wqkv[:, kb, 2 * DL:3 * DL],
                                start=False, stop=(kb == 7))
                        copy_engine().tensor_copy(
                            vb[tb][:, :, s, :, 0:64],
                            pp[:].rearrange("p (a h e) -> p a h e", a=4, h=2))
                        pp_open[0] = False
                    return [a, b]

                for fc in range(8):
                    units.extend(qk_unit(fc))
                for s in range(4):
                    units.extend(v_unit(s))
                return units

            ob = {}  # (j, pr) -> attention-output SBUF tile [128, TQ] bf16

            def wproj_pair(j, mc0):
                """Output-proj for mc0, mc0+1, kc-major: the kc=3 matmuls
                (gated on the last pair's normalize) come last, so the PE
                isn't stalled mid-unit waiting for ob tiles."""
                def run():
                    yps = [psum.tile([128, TQ], fp32, tag="pp", bufs=2,
                                     name=f"yp_{j}_{mc0 + i}")
                           for i in range(2)]
                    for kc in range(PAIRS):
                        for i in range(2):
                            mc = mc0 + i
                            nc.tensor.matmul(
                                yps[i][:], wp[:, kc, mc * 128:(mc + 1) * 128],
                                ob[(j, kc)][:],
                                start=(kc == 0), stop=(kc == PAIRS - 1))
                    for i in range(2):
                        mc = mc0 + i
                        ys = pool.tile([128, TQ], fp32, tag="ys", bufs=2,
                                       name=f"ys_{j}_{mc}")
                        nc.vector.tensor_copy(ys[:], yps[i][:])
                        nc.sync.dma_start(
                            yt_d[mc * 128:(mc + 1) * 128, j * TQ:(j + 1) * TQ],
                            ys[:])
                return run

            # ---- attention tile pipeline (AV delayed by one tile) ----
            pending = [None]

            def do_av(j, pr, c, h, half, ot, ex):
                for tkb in range(2):
                    blk = half * 2 + tkb
                    d = 128 * blk if c == j else 0
                    nc.tensor.matmul(
                        ot[:, d:TQ], vb[c][:, pr, blk, h, :],
                        ex[:, tkb, d:TQ],
                        start=(c == 0 and half == 0 and tkb == 0),
                        stop=(c == j and half == 1 and tkb == 1))

            def flush_av():
                if pending[0] is None:
                    return
                args, post = pending[0]
                pending[0] = None
                do_av(*args)
                if post is not None:
                    post()

            def attn_tile(j, pr, c, h, half, ot, post=None):
                st = psum.tile([128, 2, TQ], fp32, tag="st", bufs=2,
                               name=f"st_{j}_{pr}_{c}_{h}_{half}")
                for tkb in range(2):
                    blk = half * 2 + tkb
                    d = 128 * blk if c == j else 0
                    nc.tensor.matmul(
                        st[:, tkb, d:TQ],
                        kT[c][h * 64:(h + 1) * 64, pr, blk * 128:(blk + 1) * 128],
                        qT[j][h * 64:(h + 1) * 64, pr, d:TQ],
                        start=True, stop=True)
                ex = pool.tile([128, 2, TQ], bf16, tag="ex", bufs=4,
                               name=f"ex_{j}_{pr}_{c}_{h}_{half}")
                if c < j:
                    nc.scalar.activation(ex[:], st[:], AF.Exp, scale=0.125)
                else:
                    for tkb in range(2):
                        d = 128 * (half * 2 + tkb)
                        nc.scalar.activation(ex[:, tkb, d:TQ], st[:, tkb, d:TQ],
                                             AF.Exp, scale=0.125)
                        nc.vector.tensor_mul(ex[:, tkb, d:d + 128],
                                             ex[:, tkb, d:d + 128], msk[:])
                flush_av()
                pending[0] = ((j, pr, c, h, half, ot, ex), post)

            def normalize_pair(j, pr, ots):
                """h0/h1 chains interleaved across DVE and Pool."""
                den = [pool.tile([1, TQ], fp32, tag="den", bufs=2,
                                 name=f"den_{j}_{pr}_{h}") for h in range(2)]
                bc = [pool.tile([64, TQ], fp32, tag="bc", bufs=2,
                                name=f"bc_{j}_{pr}_{h}") for h in range(2)]
                rec = [pool.tile([64, TQ], fp32, tag="rec", bufs=2,
                                 name=f"rec_{j}_{pr}_{h}") for h in range(2)]
                for h in range(2):
                    nc.vector.tensor_copy(den[h][:], ots[h][64:65, :])
                for h in range(2):
                    nc.gpsimd.partition_broadcast(bc[h][:], den[h][:])
                for h in range(2):
                    nc.vector.reciprocal_approx_fast(rec[h][:], bc[h][:])
                for h in range(2):
                    nc.vector.tensor_mul(ob[(j, pr)][h * 64:(h + 1) * 64, :],
                                         ots[h][0:64, :], rec[h][:])

            # ---- main schedule ----
            # proj blocks 0,1 fully upfront; block j+2 paced across row j;
            # wproj rows 0..5 deferred and paced across rows 6..7.
            # DMA order: first units' operands first.
            dma_wqkv(0, 128)
            xts = {0: dma_x(0, split=True)}
            dma_wqkv(128, 512)
            dma_wqkv(512, 1024)
            dma_wqkv(1024, 3 * DL)
            nc.sync.dma_start(msk[:], msk_d[:])
            xts[1] = dma_x(1, split=True)
            nc.sync.dma_start(wp[:], wp_d.rearrange("(a p) f -> p a f", p=128))
            for tb in (0, 1):
                for u in make_units(tb, xts[tb]):
                    u()

            filler = []
            fill_emitted = [0]
            fill_tiles = 4 * 7 * 4 + 4 * 8 * 4  # attn tiles in rows 6+7
            fill_done = [0]

            for j in range(NTQ):
                if j + 2 < NTQ:
                    xts[j + 2] = dma_x(j + 2)
                    row_units = make_units(j + 2, xts[j + 2])
                else:
                    row_units = []
                n_units = len(row_units)
                row_tiles = 4 * (j + 1) * 4
                tcount = 0
                emitted = 0
                for pr in range(PAIRS):
                    ob[(j, pr)] = pool.tile(
                        [128, TQ], bf16,
                        tag=(f"ob{j}_{pr}" if j < 6 else "obx"),
                        bufs=(1 if j < 6 else 8),
                        name=f"ob_{j}_{pr}")
                    ot = [psum.tile([65, TQ], fp32, tag="ot", bufs=2,
                                    name=f"ot{h}_{pr}_{j}")
                          for h in range(2)]
                    for c in range(j + 1):
                        for (h, half) in ((0, 0), (1, 0), (0, 1), (1, 1)):
                            last = (c == j and h == 1 and half == 1)
                            post = None
                            if last:
                                def post(j=j, pr=pr, ots=tuple(ot)):
                                    normalize_pair(j, pr, ots)
                            attn_tile(j, pr, c, h, half, ot[h], post)
                            tcount += 1
                            target = math.ceil(n_units * tcount / row_tiles)
                            while emitted < target:
                                row_units[emitted]()
                                emitted += 1
                            if j >= 6:
                                fill_done[0] += 1
                                ft = math.ceil(
                                    len(filler) * fill_done[0] / fill_tiles)
                                while (fill_emitted[0] < ft
                                       and not pp_open[0]):
                                    filler[fill_emitted[0]]()
                                    fill_emitted[0] += 1
                if j <= 5:
                    for mc0 in range(0, 8, 2):
                        filler.append(wproj_pair(j, mc0))
                else:
                    flush_av()
                    for mc0 in range(0, 8, 2):
                        wproj_pair(j, mc0)()
            flush_av()
            while fill_emitted[0] < len(filler):
                filler[fill_emitted[0]]()
                fill_emitted[0] += 1

    nc.compile()
    return nc


def _get_nc():
    if "nc" not in _CACHE:
        _CACHE["nc"] = _build_nc()
    return _CACHE["nc"]


def _in_maps(x, w_qkv, w_proj):
    import ml_dtypes
    bf16 = ml_dtypes.bfloat16
    p = np.arange(128, dtype=np.int32)
    msk = (p[:, None] <= p[None, :]).astype(bf16)
    maps = []
    for c in range(N_CORES):
        b, g = c // 2, c % 2
        wq = w_qkv[:, g * DL:(g + 1) * DL]
        wk = w_qkv[:, D + g * DL:D + (g + 1) * DL]
        wv = w_qkv[:, 2 * D + g * DL:2 * D + (g + 1) * DL]
        maps.append({
            "xt": np.ascontiguousarray(x[b].T).astype(bf16),
            "wqkv": np.ascontiguousarray(
                np.concatenate([wq, wk, wv], axis=1)).astype(bf16),
            "wp": np.ascontiguousarray(w_proj[g * DL:(g + 1) * DL, :]).astype(bf16),
            "msk": msk,
        })
    return maps


def _run(x, w_qkv, w_proj, trace=False):
    from concourse.bass_utils import run_bass_kernel_spmd

    nc = _get_nc()
    res = run_bass_kernel_spmd(nc, _in_maps(x, w_qkv, w_proj),
                               core_ids=list(range(N_CORES)), trace=trace)
    outs = [res.results[c]["yt"] for c in range(N_CORES)]
    y = np.stack([(outs[2 * b] + outs[2 * b + 1]).T for b in range(B)])
    return np.ascontiguousarray(y.astype(np.float32)), res


def kernel(x, w_qkv, w_proj):
    x = np.asarray(x, dtype=np.float32)
    w_qkv = np.asarray(w_qkv, dtype=np.float32)
    w_proj = np.asarray(w_proj, dtype=np.float32)
    y, _ = _run(x, w_qkv, w_proj, trace=False)
    return y


def kernel_traced(x, w_qkv, w_proj):
    """Test-only entry: run with NTFF profiling (needs the sibling prof_shim
    module; the graded kernel() path never imports it)."""
    import prof_shim
    prof_shim.install()
    y, res = _run(np.asarray(x, np.float32), np.asarray(w_qkv, np.float32),
                  np.asarray(w_proj, np.float32), trace=True)
    return y, res


# revision 35
# speedup vs baseline: 1.0146x; 1.0032x over previous
"""Multi-head causal attention (B=4, T=4096, D=1024, H=16) on 8 TRN2 NeuronCores.

Sharding: core c -> (batch b = c//2, head-group g = c%2 of 8 heads).
Host sums the two per-batch partials (w_proj row-split) and transposes.

v2 design (vs v1 baseline at 1233us):
  - bf16 operands everywhere (matmul rate unchanged vs fp32r, but halves
    SBUF/HBM footprint); fp32 PSUM accumulation throughout.
  - Everything SBUF-resident: K^T / V / Q^T live in SBUF between the QKV
    projection and attention -- no DRAM round-trip (v1 moved ~110MB).
  - No PE transposes: host supplies x^T, projection emits Q^T/K^T directly
    (W-stationary) and V in natural layout (x^T-stationary).
  - Causal diagonal blocks trimmed at 128-granularity: S / exp / mask / AV
    restricted to valid columns (-8% PE and ScalarE work).
  - Software pipeline: AV for tile k is emitted while tile k+1's S/exp are
    in flight, so the PE never sits on the just-issued exp. Projection and
    (deferred) output-projection units are paced into the attention stream
    as PE gap fillers; ScalarE (exp, ~580us) hides under PE (~670us).
  - softmax without max-subtraction (logits ~N(0,1)); denominator via a
    ones-column in V (comes out of the same AV matmul, PSUM row 64).
"""
import math

import numpy as np

B, T, D = 4, 4096, 1024
H, HD = 16, 64
N_CORES = 8
PAIRS = 4            # head-pairs per core (8 local heads)
DL = PAIRS * 128     # 512 = local q/k/v width
TQ = 512             # query block
NTQ = T // TQ        # 8

_CACHE = {}


def _build_nc():
    import concourse.tile as tile
    from concourse import bacc, mybir

    fp32 = mybir.dt.float32
    bf16 = mybir.dt.bfloat16
    AF = mybir.ActivationFunctionType

    nc = bacc.Bacc("TRN2", target_bir_lowering=False, debug=False,
                   num_devices=N_CORES)
    xt_d = nc.dram_tensor("xt", [D, T], bf16, kind="ExternalInput").ap()
    # Q|K weights host-prearranged to [p, fc, kb, col] so each fc-slice DMA
    # is 2KB-contiguous per partition (the naive [D, cols] layout fragments
    # into 256B segments and runs ~6x under DMA bandwidth).
    wqk_d = nc.dram_tensor("wqkvqk", [128, 8, 8, 128], bf16,
                           kind="ExternalInput").ap()
    wv_d = nc.dram_tensor("wqkvv", [D, DL], bf16, kind="ExternalInput").ap()
    wp_d = nc.dram_tensor("wp", [DL, D], bf16, kind="ExternalInput").ap()
    msk_d = nc.dram_tensor("msk", [128, 128], bf16, kind="ExternalInput").ap()
    yt_d = nc.dram_tensor("yt", [D, T], fp32, kind="ExternalOutput").ap()

    with tile.TileContext(nc) as tc:
        with (
            tc.tile_pool(name="sb", bufs=1) as pool,
            tc.tile_pool(name="ps", bufs=1, space="PSUM") as psum,
        ):
            wqk = pool.tile([128, 8, 8, 128], bf16, tag="wqk")
            wv = pool.tile([128, 8, DL], bf16, tag="wv")

            def dma_wqk(fc):
                nc.sync.dma_start(wqk[:, fc, :, :], wqk_d[:, fc, :, :])

            wp = pool.tile([128, 4, D], bf16, tag="wp")
            msk = pool.tile([128, 128], bf16, tag="msk")

            # K^T and V resident for all 8 token-blocks; Q^T rotates (only
            # row j reads q block j; block j+2 is written during row j).
            kT = [pool.tile([128, PAIRS, TQ], bf16, tag=f"kT{t}",
                            name=f"kT_{t}")
                  for t in range(NTQ)]
            vb = [pool.tile([128, PAIRS, 4, 2, 65], bf16, tag=f"vb{t}",
                            name=f"vb_{t}")
                  for t in range(NTQ)]
            for t in range(NTQ):
                # ones column per head -> softmax denominator out of AV matmul
                nc.vector.memset(vb[t][:, :, :, :, 64:65], 1.0)

            qT = {}

            def dma_x(tb, split=False):
                xt = pool.tile([128, 8, TQ], bf16, tag="xt", bufs=2,
                               name=f"xt_{tb}")
                src = xt_d[:, tb * TQ:(tb + 1) * TQ]
                if split:  # halves so the first matmuls start sooner
                    nc.sync.dma_start(
                        xt[:, 0:4, :],
                        src[0:512, :].rearrange("(a p) t -> p a t", p=128))
                    nc.sync.dma_start(
                        xt[:, 4:8, :],
                        src[512:1024, :].rearrange("(a p) t -> p a t", p=128))
                else:
                    nc.sync.dma_start(
                        xt[:], src.rearrange("(a p) t -> p a t", p=128))
                return xt

            def copy_engine():
                # GPSIMD/Pool cannot read PSUM on TRN2; DVE does all
                # PSUM->SBUF drains.
                return nc.vector

            # True while a projection unit's PSUM accumulation is half-open
            # (between its a/b halves). wproj pairs must not allocate pp
            # slots in that window (in-order PE + tag rotation would
            # deadlock), so the fill pacing checks this flag.
            pp_open = [False]

            def make_units(tb, xt):
                """Projection PE-work for token rows [tb*512,+512), split
                into 4-matmul halves so paced insertions between attention
                tiles never outlast the 2-deep exp queue (~2.2us)."""
                units = []

                def qk_unit(fc):
                    st_ = {}

                    def a():
                        if fc == 0:
                            qT[tb] = pool.tile([128, PAIRS, TQ], bf16,
                                               tag="qt", bufs=3,
                                               name=f"qt_{tb}")
                        st_["pp"] = psum.tile([128, TQ], fp32, tag="pp",
                                              bufs=2, name=f"pqk_{tb}_{fc}")
                        for kb in range(4):
                            nc.tensor.matmul(
                                st_["pp"][:], wqk[:, fc, kb, :],
                                xt[:, kb, :], start=(kb == 0), stop=False)
                        pp_open[0] = True

                    def b():
                        pp = st_["pp"]
                        for kb in range(4, 8):
                            nc.tensor.matmul(
                                pp[:], wqk[:, fc, kb, :],
                                xt[:, kb, :], start=False, stop=(kb == 7))
                        dst = qT[tb] if fc < 4 else kT[tb]
                        copy_engine().tensor_copy(dst[:, fc % 4, :], pp[:])
                        pp_open[0] = False
                    return [a, b]

                def v_unit(s):
                    st_ = {}

                    def a():
                        st_["pp"] = psum.tile([128, TQ], fp32, tag="pp",
                                              bufs=2, name=f"pv_{tb}_{s}")
                        for kb in range(4):
                            nc.tensor.matmul(
                                st_["pp"][:], xt[:, kb, s * 128:(s + 1) * 128],
                                wv[:, kb, :],
                                start=(kb == 0), stop=False)
                        pp_open[0] = True

                    def b():
                        pp = st_["pp"]
                        for kb in range(4, 8):
                            nc.tensor.matmul(
                                pp[:], xt[:, kb, s * 128:(s + 1) * 128],
                                wv[:, kb, :],
                                start=False, stop=(kb == 7))
                        copy_engine().tensor_copy(
                            vb[tb][:, :, s, :, 0:64],
                            pp[:].rearrange("p (a h e) -> p a h e", a=4, h=2))
                        pp_open[0] = False
                    return [a, b]

                for fc in range(8):
                    units.extend(qk_unit(fc))
                for s in range(4):
                    units.extend(v_unit(s))
                return units

            ob = {}  # (j, pr) -> attention-output SBUF tile [128, TQ] bf16

            def wproj_pair(j, mc0):
                """Output-proj for mc0, mc0+1, kc-major: the kc=3 matmuls
                (gated on the last pair's normalize) come last, so the PE
                isn't stalled mid-unit waiting for ob tiles."""
                def run():
                    yps = [psum.tile([128, TQ], fp32, tag="pp", bufs=2,
                                     name=f"yp_{j}_{mc0 + i}")
                           for i in range(2)]
                    for kc in range(PAIRS):
                        for i in range(2):
                            mc = mc0 + i
                            nc.tensor.matmul(
                                yps[i][:], wp[:, kc, mc * 128:(mc + 1) * 128],
                                ob[(j, kc)][:],
                                start=(kc == 0), stop=(kc == PAIRS - 1))
                    for i in range(2):
                        mc = mc0 + i
                        ys = pool.tile([128, TQ], fp32, tag="ys", bufs=3,
                                       name=f"ys_{j}_{mc}")
                        if j == 7:
                            # ScalarE is idle after the last exp; keep the
                            # drain chain off the (busier) DVE queue.
                            nc.scalar.copy(ys[:], yps[i][:])
                        else:
                            nc.vector.tensor_copy(ys[:], yps[i][:])
                        nc.sync.dma_start(
                            yt_d[mc * 128:(mc + 1) * 128, j * TQ:(j + 1) * TQ],
                            ys[:])
                return run

            # ---- attention tile pipeline (AV delayed by one tile) ----
            pending = [None]

            def do_av(j, pr, c, h, half, ot, ex):
                for tkb in range(2):
                    blk = half * 2 + tkb
                    d = 128 * blk if c == j else 0
                    nc.tensor.matmul(
                        ot[:, d:TQ], vb[c][:, pr, blk, h, :],
                        ex[:, tkb, d:TQ],
                        start=(c == 0 and half == 0 and tkb == 0),
                        stop=(c == j and half == 1 and tkb == 1))

            def flush_av():
                if pending[0] is None:
                    return
                args, post = pending[0]
                pending[0] = None
                do_av(*args)
                if post is not None:
                    post()

            def attn_tile(j, pr, c, h, half, ot, post=None):
                st = psum.tile([128, 2, TQ], fp32, tag="st", bufs=2,
                               name=f"st_{j}_{pr}_{c}_{h}_{half}")
                for tkb in range(2):
                    blk = half * 2 + tkb
                    d = 128 * blk if c == j else 0
                    nc.tensor.matmul(
                        st[:, tkb, d:TQ],
                        kT[c][h * 64:(h + 1) * 64, pr, blk * 128:(blk + 1) * 128],
                        qT[j][h * 64:(h + 1) * 64, pr, d:TQ],
                        start=True, stop=True)
                ex = pool.tile([128, 2, TQ], bf16, tag="ex", bufs=4,
                               name=f"ex_{j}_{pr}_{c}_{h}_{half}")
                if c < j:
                    nc.scalar.activation(ex[:], st[:], AF.Exp, scale=0.125)
                else:
                    for tkb in range(2):
                        d = 128 * (half * 2 + tkb)
                        nc.scalar.activation(ex[:, tkb, d:TQ], st[:, tkb, d:TQ],
                                             AF.Exp, scale=0.125)
                        nc.vector.tensor_mul(ex[:, tkb, d:d + 128],
                                             ex[:, tkb, d:d + 128], msk[:])
                flush_av()
                pending[0] = ((j, pr, c, h, half, ot, ex), post)

            def normalize_pair(j, pr, ots):
                """h0/h1 chains interleaved across DVE and Pool."""
                den = [pool.tile([1, TQ], fp32, tag="den", bufs=2,
                                 name=f"den_{j}_{pr}_{h}") for h in range(2)]
                bc = [pool.tile([64, TQ], fp32, tag="bc", bufs=2,
                                name=f"bc_{j}_{pr}_{h}") for h in range(2)]
                rec = [pool.tile([64, TQ], fp32, tag="rec", bufs=2,
                                 name=f"rec_{j}_{pr}_{h}") for h in range(2)]
                for h in range(2):
                    nc.vector.tensor_copy(den[h][:], ots[h][64:65, :])
                for h in range(2):
                    nc.gpsimd.partition_broadcast(bc[h][:], den[h][:])
                for h in range(2):
                    nc.vector.reciprocal_approx_fast(rec[h][:], bc[h][:])
                for h in range(2):
                    nc.vector.tensor_mul(ob[(j, pr)][h * 64:(h + 1) * 64, :],
                                         ots[h][0:64, :], rec[h][:])

            # ---- main schedule ----
            # proj blocks 0,1 fully upfront; block j+2 paced across row j;
            # wproj rows 0..5 deferred and paced across rows 6..7.
            # DMA order: first units' operands first.
            dma_wqk(0)
            xts = {0: dma_x(0, split=True)}
            for fc in range(1, 8):
                dma_wqk(fc)
            nc.sync.dma_start(
                wv[:], wv_d.rearrange("(a p) f -> p a f", p=128))
            nc.sync.dma_start(msk[:], msk_d[:])
            xts[1] = dma_x(1, split=True)
            nc.sync.dma_start(wp[:], wp_d.rearrange("(a p) f -> p a f", p=128))
            for tb in (0, 1):
                for u in make_units(tb, xts[tb]):
                    u()

            filler = []
            fill_emitted = [0]
            fill_tiles = 4 * 7 * 4 + 4 * 8 * 4  # attn tiles in rows 6+7
            fill_done = [0]

            for j in range(NTQ):
                if j + 2 < NTQ:
                    xts[j + 2] = dma_x(j + 2)
                    row_units = make_units(j + 2, xts[j + 2])
                else:
                    row_units = []
                n_units = len(row_units)
                row_tiles = 4 * (j + 1) * 4
                tcount = 0
                emitted = 0
                for pr in range(PAIRS):
                    ob[(j, pr)] = pool.tile(
                        [128, TQ], bf16,
                        tag=(f"ob{j}_{pr}" if j < 6 else "obx"),
                        bufs=(1 if j < 6 else 8),
                        name=f"ob_{j}_{pr}")
                    ot = [psum.tile([65, TQ], fp32, tag="ot", bufs=2,
                                    name=f"ot{h}_{pr}_{j}")
                          for h in range(2)]
                    for c in range(j + 1):
                        for (h, half) in ((0, 0), (1, 0), (0, 1), (1, 1)):
                            last = (c == j and h == 1 and half == 1)
                            post = None
                            if last:
                                def post(j=j, pr=pr, ots=tuple(ot)):
                                    normalize_pair(j, pr, ots)
                            attn_tile(j, pr, c, h, half, ot[h], post)
                            tcount += 1
                            target = math.ceil(n_units * tcount / row_tiles)
                            while emitted < target:
                                row_units[emitted]()
                                emitted += 1
                            if j >= 6:
                                fill_done[0] += 1
                                ft = math.ceil(
                                    len(filler) * fill_done[0] / fill_tiles)
                                while (fill_emitted[0] < ft
                                       and not pp_open[0]):
                                    filler[fill_emitted[0]]()
                                    fill_emitted[0] += 1
                if j <= 5:
                    for mc0 in range(0, 8, 2):
                        filler.append(wproj_pair(j, mc0))
                else:
                    flush_av()
                    for mc0 in range(0, 8, 2):
                        wproj_pair(j, mc0)()
            flush_av()
            while fill_emitted[0] < len(filler):
                filler[fill_emitted[0]]()
                fill_emitted[0] += 1

    nc.compile()
    return nc


def _get_nc():
    if "nc" not in _CACHE:
        _CACHE["nc"] = _build_nc()
    return _CACHE["nc"]


def _in_maps(x, w_qkv, w_proj):
    import ml_dtypes
    bf16 = ml_dtypes.bfloat16
    p = np.arange(128, dtype=np.int32)
    msk = (p[:, None] <= p[None, :]).astype(bf16)
    maps = []
    for c in range(N_CORES):
        b, g = c // 2, c % 2
        wq = w_qkv[:, g * DL:(g + 1) * DL]
        wk = w_qkv[:, D + g * DL:D + (g + 1) * DL]
        wv = w_qkv[:, 2 * D + g * DL:2 * D + (g + 1) * DL]
        # [D, 1024] -> [p, fc, kb, col] (2KB-contiguous fc-slices for DMA)
        wqk = np.concatenate([wq, wk], axis=1).reshape(8, 128, 8, 128)
        wqk = np.ascontiguousarray(wqk.transpose(1, 2, 0, 3))
        maps.append({
            "xt": np.ascontiguousarray(x[b].T).astype(bf16),
            "wqkvqk": wqk.astype(bf16),
            "wqkvv": np.ascontiguousarray(wv).astype(bf16),
            "wp": np.ascontiguousarray(w_proj[g * DL:(g + 1) * DL, :]).astype(bf16),
            "msk": msk,
        })
    return maps


def _run(x, w_qkv, w_proj, trace=False):
    from concourse.bass_utils import run_bass_kernel_spmd

    nc = _get_nc()
    res = run_bass_kernel_spmd(nc, _in_maps(x, w_qkv, w_proj),
                               core_ids=list(range(N_CORES)), trace=trace)
    outs = [res.results[c]["yt"] for c in range(N_CORES)]
    y = np.stack([(outs[2 * b] + outs[2 * b + 1]).T for b in range(B)])
    return np.ascontiguousarray(y.astype(np.float32)), res


def kernel(x, w_qkv, w_proj):
    x = np.asarray(x, dtype=np.float32)
    w_qkv = np.asarray(w_qkv, dtype=np.float32)
    w_proj = np.asarray(w_proj, dtype=np.float32)
    y, _ = _run(x, w_qkv, w_proj, trace=False)
    return y


def kernel_traced(x, w_qkv, w_proj):
    """Test-only entry: run with NTFF profiling (needs the sibling prof_shim
    module; the graded kernel() path never imports it)."""
    import prof_shim
    prof_shim.install()
    y, res = _run(np.asarray(x, np.float32), np.asarray(w_qkv, np.float32),
                  np.asarray(w_proj, np.float32), trace=True)
    return y, res
